# revision 9
# baseline (speedup 1.0000x reference)
"""DigitalCapsule dynamic-routing kernel for 8 TRN2 NeuronCores.

Math (per batch b, out-capsule n):
    u_hat[p,d] = sum_e x[b,p,e] W[n,p,e,d]
    3 routing iters: c = softmax_p(logits), s = sum_p c*u_hat,
    v = squash(s), logits += v . u_hat
Output v: [B, N, D],  B=128, N=32, P=1152, E=8, D=16.

Sharding: N across the 8 cores (4 capsules each), full B per core.
Routing is independent per n, so there is no cross-core communication.

Formulation avoids materializing u_hat (302 MB):
  s-step:  s[b,d]   = sum_{pe} Y[b,pe] W[pe,d],  Y = exp(logit)*x (bf16)
  logits:  G[pe,b]  = sum_d W[pe,d] v[b,d]        (PE, fp32 "float32r")
           a[p,b]  += sum_e xT[pe,b]*G[pe,b]      (fp16 mul on DVE,
                                                   e-sum via identity
                                                   matmuls into PSUM)
All contractions run on the tensor engine; PSUM accumulates in fp32.
v is kept in [d, b] layout throughout, so no transposes are needed.
"""

import numpy as np
import ml_dtypes

B, N, P, E, D = 128, 32, 1152, 8, 16
NCORES = 8
NS = N // NCORES          # capsules per core
PEF = P * E               # 9216 flattened (e, p) contraction dim
T = PEF // 128            # 72 K-tiles
PC = P // 128             # 9 p-chunks
EPS = 1e-8

_COMPILED = None


def _build():
    import concourse.bass as bass
    import concourse.tile as tile
    from concourse import bacc, mybir

    f32, f32r = mybir.dt.float32, mybir.dt.float32r
    f16, bf16 = mybir.dt.float16, mybir.dt.bfloat16
    mult = mybir.AluOpType.mult
    Act = mybir.ActivationFunctionType

    nc = bacc.Bacc("TRN2", target_bir_lowering=False)

    xTh_d = nc.dram_tensor("xTh", [128, T, 128], f16, kind="ExternalInput")
    xTb_d = nc.dram_tensor("xTb", [128, T, 128], bf16, kind="ExternalInput")
    W4b_d = nc.dram_tensor("W4b", [128, T, NS * 32], bf16, kind="ExternalInput")
    WT_d = nc.dram_tensor("WT32", [NS * 32, PEF], f32r, kind="ExternalInput")
    Ir_d = nc.dram_tensor("I128r", [128, 128], f32r, kind="ExternalInput")
    z_d = nc.dram_tensor("zblk", [NS * 32, NS * 128], f32r, kind="ExternalInput")
    o16_d = nc.dram_tensor("ones16r", [16, 1], f32r, kind="ExternalInput")
    I_d = nc.dram_tensor("I128h", [128, 128], f16, kind="ExternalInput")
    out_d = nc.dram_tensor("out4", [NS, D, 128], f32, kind="ExternalOutput")

    with tile.TileContext(nc) as tc:
        _emit(tc, nc, bass, mybir, xTh_d, xTb_d, W4b_d, WT_d, I_d, Ir_d, z_d, o16_d, out_d,
              f32, f32r, f16, bf16, mult, Act)
    nc.compile()
    return nc


def _emit(tc, nc, bass, mybir, xTh_d, xTb_d, W4b_d, WT_d, I_d, Ir_d, z_d, o16_d, out_d,
          f32, f32r, f16, bf16, mult, Act):
    from contextlib import ExitStack
    ctx = ExitStack()
    singles = ctx.enter_context(tc.tile_pool(name="singles", bufs=1))
    gpool = ctx.enter_context(tc.tile_pool(name="gth", bufs=2))
    ppool = ctx.enter_context(tc.tile_pool(name="prod", bufs=2))
    ypool = ctx.enter_context(tc.tile_pool(name="yt", bufs=2))
    small = ctx.enter_context(tc.tile_pool(name="small", bufs=4))
    ps_big = ctx.enter_context(tc.tile_pool(name="psb", bufs=2, space="PSUM"))
    ps_a = ctx.enter_context(tc.tile_pool(name="psa", bufs=2, space="PSUM"))
    ps_s = ctx.enter_context(tc.tile_pool(name="pss", bufs=1, space="PSUM"))

    # --- persistent SBUF tensors ---
    xTh = singles.tile([128, T, 128], f16)
    xTb = singles.tile([128, T, 128], bf16)
    W4b = singles.tile([128, T, NS * 32], bf16)
    WT = singles.tile([NS * 32, PEF], f32r)
    I128h = singles.tile([128, 128], f16)
    I128r = singles.tile([128, 128], f32r)
    vblk = singles.tile([NS * 32, NS * 128], f32r)      # block-diag v, [d,b] slots
    aacc = singles.tile([128, PC, NS * 128], f32r)     # saved iter-2 logits (T-layout)
    expb = singles.tile([128, PC, NS * 128], bf16)    # exp(logits)
    rd = singles.tile([1, NS * 128], f32)             # 1/den per (n,b)
    ones16 = singles.tile([16, 1], f32r)
    ones128b = singles.tile([128, 1], bf16)
    eps1 = singles.tile([1, 1], f32)
    one1 = singles.tile([1, 1], f32)

    nc.sync.dma_start(xTh, xTh_d.ap())
    nc.sync.dma_start(xTb, xTb_d.ap())
    nc.sync.dma_start(W4b, W4b_d.ap())
    nc.sync.dma_start(WT, WT_d.ap())
    nc.sync.dma_start(I128h, I_d.ap())
    nc.sync.dma_start(I128r, Ir_d.ap())
    nc.sync.dma_start(vblk, z_d.ap())
    nc.sync.dma_start(ones16, o16_d.ap())
    nc.vector.memset(ones128b, 1.0)
    nc.vector.memset(eps1, EPS)
    nc.vector.memset(one1, 1.0)

    xTb_e = xTb.rearrange("p (e c) b -> p e c b", c=PC)   # [128, 8, 9, 128]
    xTh_e = xTh.rearrange("p (e c) b -> p e c b", c=PC)

    def squash_to(n, s_ps, it):
        """s_ps: PSUM [16, 128] = s (unnormalized) for capsule n.
        Writes v in [d,b] layout to vblk slot (it<3) or DMA out (it==3)."""
        t16 = small.tile([16, 128], f32)
        if it == 1:
            nc.vector.tensor_scalar_mul(t16, s_ps, 1.0 / P)
        else:
            rd16 = small.tile([16, 128], f32)
            src = rd[:, n * 128:(n + 1) * 128]
            bc = bass.AP(tensor=src.tensor, offset=src.offset,
                         ap=[list(src.ap[0]), [0, 16]] + list(src.ap[1:]))
            nc.gpsimd.dma_start(out=rd16, in_=bc)
            nc.vector.tensor_mul(t16, s_ps, rd16)
        t2 = small.tile([16, 128], f32r)
        nc.vector.tensor_mul(t2, t16, t16)
        sq_ps = ps_s.tile([1, 128], f32)
        nc.tensor.matmul(sq_ps, lhsT=ones16, rhs=t2, start=True, stop=True)
        sqs = small.tile([1, 128], f32)
        nc.vector.tensor_copy(sqs, sq_ps)
        w1 = small.tile([1, 128], f32)
        nc.scalar.activation(w1, sqs, Act.Sqrt, bias=eps1)        # sqrt(sq+eps)
        w2 = small.tile([1, 128], f32)
        nc.vector.tensor_scalar_add(w2, sqs, 1.0)                 # 1+sq
        nc.vector.tensor_mul(w2, w2, w1)                          # (1+sq)*sqrt(..)
        nc.vector.reciprocal(w1, w2)
        nc.vector.tensor_mul(w1, w1, sqs)                         # sq/((1+sq)sqrt(..))
        sc16 = small.tile([16, 128], f32)
        bc = bass.AP(tensor=w1.tensor, offset=w1.offset,
                     ap=[list(w1.ap[0]), [0, 16]] + list(w1.ap[1:]))
        nc.gpsimd.dma_start(out=sc16, in_=bc)
        if it == 3:
            vout = small.tile([16, 128], f32)
            nc.vector.tensor_mul(vout, t16, sc16)
            nc.sync.dma_start(out_d.ap()[n], vout)
        else:
            nc.vector.tensor_mul(
                vblk[n * 32:n * 32 + 16, n * 128:(n + 1) * 128], t16, sc16)

    # ---------- iteration 1: s1 = (1/P) sum_pe x W  (all 4 n at once) ----------
    s4_ps = ps_s.tile([NS * 32, 128], f32)
    for t in range(T):
        nc.tensor.matmul(s4_ps, lhsT=W4b[:, t, :], rhs=xTb[:, t, :],
                         start=(t == 0), stop=(t == T - 1))
    for n in range(NS):
        squash_to(n, s4_ps[n * 32:n * 32 + 16, :], 1)

    # ---------- iterations 2, 3 ----------
    for it in (2, 3):
        # logits update: G = W v (PE), prod = xT*G (fp16), e-sum (PE)
        for pc in range(PC):
            aT_ps = ps_a.tile([128, NS * 128], f32)
            first_mm = True
            if it == 3:
                nc.tensor.matmul(aT_ps, lhsT=I128r,
                                 rhs=aacc[:, pc, :],
                                 start=True, stop=False, skip_group_check=True)
                first_mm = False
            gt = gpool.tile([128, E, NS * 128], f16)
            for e in range(E):
                t_idx = e * PC + pc
                g_ps = ps_big.tile([128, NS * 128], f32)
                nc.tensor.matmul(
                    g_ps,
                    lhsT=WT[:, t_idx * 128:(t_idx + 1) * 128],
                    rhs=vblk, start=True, stop=True)
                nc.scalar.activation(gt[:, e, :], g_ps, Act.Copy)
            prod = ppool.tile([128, E, NS * 128], f16)
            xe = xTh_e[:, :, pc, :]                     # [128, 8, 128]
            xeb = xe[:, :, None, :].to_broadcast([128, E, NS, 128])
            nc.vector.tensor_tensor(
                prod.rearrange("p e (n b) -> p e n b", n=NS),
                gt.rearrange("p e (n b) -> p e n b", n=NS), xeb, mult)
            for e in range(E):
                nc.tensor.matmul(aT_ps, lhsT=I128h, rhs=prod[:, e, :],
                                 start=(first_mm and e == 0), stop=(e == E - 1),
                                 skip_group_check=True)
            if it == 2:
                nc.vector.tensor_copy(aacc[:, pc, :], aT_ps)
            nc.scalar.activation(expb[:, pc, :], aT_ps, Act.Exp)
        # denominators (sum over p = partition dim, via ones matmul)
        den_ps = ps_s.tile([1, NS * 128], f32)
        for pc in range(PC):
            nc.tensor.matmul(den_ps, lhsT=ones128b, rhs=expb[:, pc, :],
                             start=(pc == 0), stop=(pc == PC - 1))
        nc.vector.reciprocal(rd, den_ps)
        # per-capsule: Y = exp * x (bf16), s = sum_pe Y W, squash
        for n in range(NS):
            yt = ypool.tile([128, T, 128], bf16)
            en = expb[:, :, n * 128:(n + 1) * 128]       # [128, 9, 128]
            enb = en[:, None, :, :].to_broadcast([128, E, PC, 128])
            nc.vector.tensor_tensor(
                yt.rearrange("p (e c) b -> p e c b", c=PC), xTb_e, enb, mult)
            sn_full = ps_s.tile([NS * 32, 128], f32)
            sn_ps = sn_full[:D]
            for t in range(T):
                nc.tensor.matmul(sn_ps,
                                 lhsT=W4b[:, t, n * 32:n * 32 + 16],
                                 rhs=yt[:, t, :],
                                 start=(t == 0), stop=(t == T - 1))
            squash_to(n, sn_ps, it)
    ctx.close()


def _host_prep(x, W):
    """Per-core input arrays (layout-only transforms)."""
    xT = np.ascontiguousarray(x.transpose(2, 1, 0)).reshape(PEF, B)  # (e,p),b
    xT_t = np.ascontiguousarray(xT.reshape(T, 128, B).transpose(1, 0, 2))
    xTh = xT_t.astype(np.float16)
    xTb = xT_t.astype(ml_dtypes.bfloat16)
    ident = np.eye(128, dtype=np.float16)
    identr = np.eye(128, dtype=np.float32)
    maps = []
    for r in range(NCORES):
        Ws = W[r * NS:(r + 1) * NS]                      # [4, P, E, D]
        Wp = np.zeros((NS, P, E, 32), np.float32)        # pad d 16->32
        Wp[:, :, :, :D] = Ws
        W4 = Wp.transpose(2, 1, 0, 3).reshape(PEF, NS * 32)  # [(e,p),(n,dpad)]
        W4b = np.ascontiguousarray(
            W4.reshape(T, 128, NS * 32).transpose(1, 0, 2)).astype(ml_dtypes.bfloat16)
        WT32 = np.ascontiguousarray(
            Wp.transpose(0, 3, 2, 1).reshape(NS * 32, PEF))    # [(n,dpad),(e,p)]
        maps.append({"xTh": xTh, "xTb": xTb, "W4b": W4b,
                     "WT32": WT32, "I128h": ident, "I128r": identr,
                     "zblk": np.zeros((NS * 32, NS * 128), np.float32),
                     "ones16r": np.ones((16, 1), np.float32)})
    return maps


def kernel(x, W):
    global _COMPILED
    from concourse import bass_utils
    if _COMPILED is None:
        _COMPILED = _build()
    in_maps = _host_prep(np.asarray(x, np.float32), np.asarray(W, np.float32))
    res = bass_utils.run_bass_kernel_spmd(
        _COMPILED, in_maps, core_ids=list(range(NCORES)))
    out = np.empty((B, N, D), np.float32)
    for r in range(NCORES):
        o = res.results[r]["out4"]                       # [4, 16, 128]
        out[:, r * NS:(r + 1) * NS, :] = np.asarray(o).transpose(2, 0, 1)
    return out
